# revision 36
# baseline (speedup 1.0000x reference)
"""Trainium2 raw-bass kernel for nn_Connection_v5 (geodesic-spray RHS).

Math (per sample n, D=128, 2D=256):
    x = input_[:, :D], v = input_[:, D:]
    z1 = x @ W1.T + b1 ; h = relu(z1) ; mask = z1 > 0
    s  = sigmoid(h @ W2.T + b2) ; sign_j = -1 if j < 4 else 1
    g  = (s + 0.618) * sign ; jac[i,j] = sign_i s_i(1-s_i) (W2 (mask*W1))[i,j]
    dv[j] = -1/g_j sum_i v_i^2 jac[i,j] + 2 v_j/g_j sum_i v_i jac[j,i]
    out = [v, dv]

Folded all-bf16 dataflow (host-validated ~7e-3 rel err vs the 2e-2 gate):
    [z1|u]^T = W1T @ [x|v]^T                    M1M3 (bf16, 512 moving)
    h = relu(z1+b1) ; s = sigmoid(z2+b2)        scalar ACT from PSUM
    nsps=(s-1)s ; wt=v^2*nsps ; gr=1/(s+.618) ; grw=gr*v*nsps
    at = (sign_i*W2)^T wt                       M4 (overwrites z1 in PSUM)
    mu = (h>0)*u ; am = (h>0)*at                two 3D DVE ops
    At'= (W1*sign_j) @ am ; Ct = (-2 W2^T) @ mu M5/M6 -> one PSUM tile
    dv = gr*At' + grw*Ct                        paired mult + add
    dv^T -> sample-major via PE transpose, stored bf16, widened on host

Implementation: raw bass (no TileContext).  Five hand-written in-order
engine programs in a 7-iteration software pipeline (4 chunks of 256
samples), manual semaphores, statically allocated per-chunk SBUF (no reuse
-> no WAR sems), hand-packed PSUM (8 banks exactly).  Inputs are
pre-transposed to feature-major bf16 on the host so each chunk's DMA lands
directly as a matmul moving operand.  The PE is warmed with junk matmuls
while the first input chunk + weights are in flight.  Ends with a single
block barrier + sem clear (vs Tile's ~10us drain butterfly).

Sharding: pure data-parallel over N=8192 across 8 cores; weights
replicated; the v half of the output never touches the device.
"""

import os
import numpy as np

D = 128
TWO_D = 256
N_TOTAL = 8192
NCORES = 8
N_CORE = N_TOTAL // NCORES  # 1024
NF = 256                    # samples per pipeline chunk
CONST = 0.618
SIGN = 4
N_WARM_MM = 6               # junk matmuls ramping PE during initial DMA wait

_CACHE = {}


def _build(n_core=N_CORE):
    from contextlib import ExitStack

    import concourse.bacc as bacc
    import concourse.mybir as mybir

    f32 = mybir.dt.float32
    bf16 = mybir.dt.bfloat16
    i32 = mybir.dt.int32
    Act = mybir.ActivationFunctionType
    Op = mybir.AluOpType

    nchunk = n_core // NF
    nb = NF // 128

    nc = bacc.Bacc("TRN2", target_bir_lowering=False, debug=False,
                   num_devices=NCORES)

    # inp[c, j, fb, n] = input row (c*NF + n), feature (fb*128 + j), bf16
    # feature-major (fb=0 -> x^T, fb=1 -> v^T), 1KB/partition/chunk.
    inp = nc.dram_tensor("inp", [nchunk, 128, 2, NF], bf16,
                         kind="ExternalInput").ap()
    wpack = nc.dram_tensor("wpack", [128, 643], f32,
                           kind="ExternalInput").ap()
    outd = nc.dram_tensor("outd", [nchunk, 128, nb, D], bf16,
                          kind="ExternalOutput").ap()

    NIT = nchunk + 3

    with ExitStack() as ctx:
        E = ctx.enter_context

        # --- SBUF (static; per-chunk copies -> no cross-chunk WAR) ---
        sb_pack = E(nc.sbuf_tensor("sb_pack", [128, 643], f32))
        warm = E(nc.sbuf_tensor("warm", [128, 1], f32))
        idn_i = E(nc.sbuf_tensor("idn_i", [128, 128], i32))
        idn_b = E(nc.sbuf_tensor("idn_b", [128, 128], bf16))
        junk = E(nc.sbuf_tensor("junk", [128, 512], bf16))
        xv = [E(nc.sbuf_tensor(f"xv{c}", [128, 2, NF], bf16))
              for c in range(nchunk)]
        h = [E(nc.sbuf_tensor(f"h{c}", [128, 2, NF], bf16))
             for c in range(nchunk)]
        s_ = [E(nc.sbuf_tensor(f"s{c}", [128, NF], bf16))
              for c in range(nchunk)]
        gs = [E(nc.sbuf_tensor(f"gs{c}", [128, NF], f32))
              for c in range(nchunk)]
        nsps = [E(nc.sbuf_tensor(f"nsps{c}", [128, NF], bf16))
                for c in range(nchunk)]
        v2 = [E(nc.sbuf_tensor(f"v2{c}", [128, NF], bf16))
              for c in range(nchunk)]
        wt = [E(nc.sbuf_tensor(f"wt{c}", [128, NF], bf16))
              for c in range(nchunk)]
        w = [E(nc.sbuf_tensor(f"w{c}", [128, NF], bf16))
             for c in range(nchunk)]
        grgw = [E(nc.sbuf_tensor(f"grgw{c}", [128, 2, NF], f32))
                for c in range(nchunk)]
        amu = [E(nc.sbuf_tensor(f"amu{c}", [128, 2, 2, NF], bf16))
               for c in range(nchunk)]
        prod = [E(nc.sbuf_tensor(f"prod{c}", [128, 2, NF], bf16))
                for c in range(nchunk)]
        dvT = [E(nc.sbuf_tensor(f"dvT{c}", [128, NF], bf16))
               for c in range(nchunk)]
        ob = [E(nc.sbuf_tensor(f"ob{c}", [128, nb, D], bf16))
              for c in range(nchunk)]

        # --- PSUM: 8 banks exactly ---
        zu = [E(nc.psum_tensor(f"zu{i}", [128, 2, 2, NF], f32))
              for i in range(2)]                            # 2 banks each
        z2 = E(nc.psum_tensor("z2", [128, NF], f32))        # 1 bank
        acp = E(nc.psum_tensor("ac", [128, 2, NF], f32))    # 1 bank
        dvp = E(nc.psum_tensor("dvp", [128, nb, D], bf16))  # 1 bank
        pat = E(nc.psum_tensor("pat", [128, 2, NF], f32))   # 1 bank (M4 out
        #     + PE warmup target; single-buffered, guarded by s_am)

        w1t = sb_pack[:, 0:128].bitcast(bf16)
        w2t = sb_pack[:, 128:256].bitcast(bf16).rearrange(
            "p (a b) -> p a b", a=2)
        w2sgn = sb_pack[:, 256:384].bitcast(bf16)
        w1p = sb_pack[:, 384:512].bitcast(bf16).rearrange(
            "p (a b) -> p a b", a=2)
        w2t2 = sb_pack[:, 512:640].bitcast(bf16).rearrange(
            "p (a b) -> p a b", a=2)
        bcol = sb_pack[:, 640:643]

        if True:
            S = {name: nc.alloc_semaphore(name) for name in
                 ["s_w1", "s_wr", "s_out",
                  "s_zu", "s_z2", "s_at", "s_ac", "s_dvp",
                  "s_h", "s_s", "s_gs", "s_ob",
                  "s_mu", "s_am", "s_nsps", "s_wt", "s_gr", "s_prod",
                  "s_v2", "s_grw", "s_dvT",
                  "s_iota", "s_jm", "s_idnb", "s_wm"]}
            s_in = [nc.alloc_semaphore(f"s_in{c}") for c in range(nchunk)]
            sem_nums = sorted(h.num for h in
                              list(S.values()) + list(s_in))
            sem_range = range(sem_nums[0], sem_nums[-1] + 1)

            NI = nchunk + 4

            with nc.Block(no_gpsimd_drain=True) as block:

                @block.sync
                def _(sp):
                    sp.dma_start(out=sb_pack[:, 0:128],
                                 in_=wpack[:, 0:128]).then_inc(S["s_w1"], 16)
                    sp.dma_start(out=xv[0][:], in_=inp[0]).then_inc(
                        s_in[0], 16)
                    sp.dma_start(out=sb_pack[:, 128:643],
                                 in_=wpack[:, 128:643]).then_inc(
                                     S["s_wr"], 16)
                    for c in range(1, nchunk):
                        sp.dma_start(out=xv[c][:], in_=inp[c]).then_inc(
                            s_in[c], 16)
                    for c in range(nchunk):
                        sp.wait_ge(S["s_ob"], c + 1)
                        sp.dma_start(out=outd[c], in_=ob[c][:]).then_inc(
                            S["s_out"], 16)
                    sp.wait_ge(S["s_out"], 16 * nchunk)

                @block.gpsimd
                def _(gp):
                    gp.iota(idn_i[:], pattern=[[1, 128]], base=0,
                            channel_multiplier=-1).then_inc(S["s_iota"])
                    gp.memset(junk[:], 0.0).then_inc(S["s_jm"])
                    for i in range(NI):
                        c = i - 1
                        if 0 <= c < nchunk:           # wt(c) = v2*nsps halves
                            gp.wait_ge(S["s_v2"], c + 1)
                            for hh in range(2):
                                gp.wait_ge(S["s_nsps"], 2 * c + hh + 1)
                                sl = slice(128 * hh, 128 * (hh + 1))
                                gp.tensor_tensor(
                                    wt[c][:, sl], v2[c][:, sl],
                                    nsps[c][:, sl],
                                    Op.mult).then_inc(S["s_wt"])
                        if 0 <= c < nchunk:           # w(c) = v*nsps
                            gp.tensor_tensor(w[c][:], xv[c][:, 1, :],
                                             nsps[c][:], Op.mult)
                        c = i - 2
                        if 0 <= c < nchunk:           # dvT(c) = prod0+prod1
                            gp.wait_ge(S["s_prod"], c + 1)
                            gp.tensor_tensor(dvT[c][:], prod[c][:, 0, :],
                                             prod[c][:, 1, :],
                                             Op.add).then_inc(S["s_dvT"])
                        c = i - 1
                        if 0 <= c < nchunk:           # grw(c) = w*gr
                            gp.wait_ge(S["s_gr"], c + 1)
                            gp.drain()   # Pool pipe flush before reading w
                            gp.tensor_tensor(grgw[c][:, 1, :], w[c][:],
                                             grgw[c][:, 0, :],
                                             Op.mult).then_inc(S["s_grw"])

                @block.vector
                def _(ve):
                    ve.memset(warm[:], 0.0).then_inc(S["s_wm"])
                    ve.wait_ge(S["s_iota"], 1)
                    ve.tensor_scalar(out=idn_b[:], in0=idn_i[:], scalar1=0,
                                     scalar2=None,
                                     op0=Op.is_equal).then_inc(S["s_idnb"])
                    for i in range(NI):
                        c = i - 2
                        if 0 <= c < nchunk:           # am(c) = (h>0)*at
                            ve.wait_ge(S["s_at"], c + 1)
                            ve.scalar_tensor_tensor(
                                out=amu[c][:, :, 0, :], in0=h[c][:],
                                scalar=0.0, in1=pat[:],
                                op0=Op.is_gt,
                                op1=Op.mult).then_inc(S["s_am"])
                        c = i - 1
                        if 0 <= c < nchunk:
                            # nsps(c) = (s-1)*s, halves
                            for hh in range(2):
                                ve.wait_ge(S["s_s"], 2 * c + hh + 1)
                                sl = slice(128 * hh, 128 * (hh + 1))
                                ve.scalar_tensor_tensor(
                                    out=nsps[c][:, sl], in0=s_[c][:, sl],
                                    scalar=1.0, in1=s_[c][:, sl],
                                    op0=Op.subtract,
                                    op1=Op.mult).then_inc(S["s_nsps"])
                            # gr(c) = 1/(s+0.618)
                            ve.wait_ge(S["s_gs"], c + 1)
                            ve.reciprocal_approx_fast(
                                out=grgw[c][:, 0, :],
                                in_=gs[c][:]).then_inc(S["s_gr"])
                        c = i - 2
                        if 0 <= c < nchunk:           # prod = grgw*[At|Ct]
                            ve.wait_ge(S["s_ac"], c + 1)
                            ve.wait_ge(S["s_grw"], c + 1)
                            ve.tensor_tensor(prod[c][:], grgw[c][:],
                                             acp[:],
                                             Op.mult).then_inc(S["s_prod"])
                        c = i
                        if 0 <= c < nchunk:           # mu(c) = (h>0)*u
                            ve.wait_ge(S["s_zu"], c + 1)
                            ve.wait_ge(S["s_h"], c + 1)
                            ve.scalar_tensor_tensor(
                                out=amu[c][:, :, 1, :], in0=h[c][:],
                                scalar=0.0, in1=zu[c % 2][:, :, 1, :],
                                op0=Op.is_gt,
                                op1=Op.mult).then_inc(S["s_mu"])

                @block.scalar
                def _(sc):
                    sc.wait_ge(S["s_wm"], 1)
                    sc.activation(out=warm[:], in_=warm[:], func=Act.Sigmoid,
                                  bias=0.0, scale=1.0)
                    sc.wait_ge(S["s_wr"], 16)   # bias columns loaded
                    for i in range(NI):
                        c = i - 4
                        if 0 <= c < nchunk:           # ob(c)
                            sc.wait_ge(S["s_dvp"], c + 1)
                            sc.copy(out=ob[c][:],
                                    in_=dvp[:]).then_inc(S["s_ob"])
                        c = i
                        if 0 <= c < nchunk:           # v2(c) = v^2 (Square)
                            sc.wait_ge(s_in[c], 16)
                            sc.activation(out=v2[c][:], in_=xv[c][:, 1, :],
                                          func=Act.Square, bias=0.0,
                                          scale=1.0).then_inc(S["s_v2"])
                        c = i - 1
                        if 0 <= c < nchunk:           # sigmoid(c) halves, gs
                            sc.wait_ge(S["s_z2"], c + 1)
                            for hh in range(2):
                                sl = slice(128 * hh, 128 * (hh + 1))
                                sc.activation(out=s_[c][:, sl],
                                              in_=z2[:, sl],
                                              func=Act.Sigmoid,
                                              bias=bcol[:, 2:3],
                                              scale=1.0).then_inc(S["s_s"])
                            sc.drain()   # ACT pipe flush before reading s
                            sc.activation(out=gs[c][:], in_=s_[c][:],
                                          func=Act.Copy, bias=CONST,
                                          scale=1.0).then_inc(S["s_gs"])
                        c = i
                        if 0 <= c < nchunk:           # relu(c)
                            sc.wait_ge(S["s_zu"], c + 1)
                            for k in range(2):
                                ins = sc.activation(
                                    out=h[c][:, k, :],
                                    in_=zu[c % 2][:, k, 0, :],
                                    func=Act.Relu,
                                    bias=bcol[:, k:k + 1], scale=1.0)
                            ins.then_inc(S["s_h"])

                @block.tensor
                def _(pe):
                    pe.wait_ge(S["s_idnb"], 1)
                    pe.wait_ge(S["s_jm"], 1)
                    for wi in range(N_WARM_MM):
                        nc.tensor.matmul(
                            pat[:].rearrange("p a b -> p (a b)"),
                            idn_b[:], junk[:], start=True, stop=True)
                    pe.wait_ge(S["s_wr"], 16)
                    for i in range(NI):
                        c = i - 2
                        if 0 <= c < nchunk:           # M4(c): at = W2sgn @ wt
                            if c >= 1:                # pat bank reuse
                                pe.wait_ge(S["s_am"], c)
                            for hh in range(2):       # moving halves
                                pe.wait_ge(S["s_wt"], 2 * c + hh + 1)
                                sl = slice(128 * hh, 128 * (hh + 1))
                                for k in range(2):
                                    ins = nc.tensor.matmul(
                                        pat[:, k, sl],
                                        w2sgn[:, 128 * k:128 * (k + 1)],
                                        wt[c][:, sl],
                                        start=True, stop=True)
                            ins.then_inc(S["s_at"])
                        c = i - 1
                        if 0 <= c < nchunk:           # M2(c)
                            pe.wait_ge(S["s_h"], c + 1)
                            if c >= 1:                # z2 bank reuse
                                pe.wait_ge(S["s_s"], 2 * c)
                            for k in range(2):
                                ins = nc.tensor.matmul(
                                    z2[:], w2t[:, k, :], h[c][:, k, :],
                                    start=(k == 0), stop=(k == 1))
                            ins.then_inc(S["s_z2"])
                        c = i
                        if 0 <= c < nchunk:           # M1M3(c)
                            pe.wait_ge(s_in[c], 16)
                            if c == 0:
                                pe.wait_ge(S["s_w1"], 16)
                            if c >= 2:                # zu slot reuse
                                pe.wait_ge(S["s_mu"], c - 1)
                                pe.wait_ge(S["s_h"], c - 1)
                            mov = xv[c][:].rearrange("p a b -> p (a b)")
                            for k in range(2):
                                ins = nc.tensor.matmul(
                                    zu[c % 2][:, k, :, :].rearrange(
                                        "p a b -> p (a b)"),
                                    w1t[:, 128 * k:128 * (k + 1)],
                                    mov, start=True, stop=True)
                            ins.then_inc(S["s_zu"])
                        c = i - 2
                        if 0 <= c < nchunk:           # M5(c), M6(c)
                            pe.wait_ge(S["s_am"], c + 1)
                            pe.wait_ge(S["s_mu"], c + 1)
                            if c >= 1:                # ac bank reuse
                                pe.wait_ge(S["s_prod"], c)
                            for k in range(2):
                                nc.tensor.matmul(
                                    acp[:, 0, :], w1p[:, k, :],
                                    amu[c][:, k, 0, :],
                                    start=(k == 0), stop=(k == 1))
                            for k in range(2):
                                ins = nc.tensor.matmul(
                                    acp[:, 1, :], w2t2[:, k, :],
                                    amu[c][:, k, 1, :],
                                    start=(k == 0), stop=(k == 1))
                            ins.then_inc(S["s_ac"])
                        c = i - 3
                        if 0 <= c < nchunk:           # trout(c)
                            pe.wait_ge(S["s_dvT"], c + 1)
                            if c >= 1:                # dv bank reuse
                                pe.wait_ge(S["s_ob"], c)
                            for b in range(nb):
                                ins = nc.tensor.transpose(
                                    dvp[:, b, :],
                                    dvT[c][:, 128 * b:128 * (b + 1)],
                                    idn_b[:])
                            ins.then_inc(S["s_dvp"])

            # after the block-exit barrier: zero our sems so a re-execution
            # of this NEFF (or a later kernel) starts clean.  Cheap
            # (~0.2us) vs Tile's dma_reset + double-barrier butterfly.
            nc.gpsimd.sem_clear(sem_range)

    nc.compile()
    return nc


def _get_nc(n_core=N_CORE):
    key = ("nc", n_core)
    if key not in _CACHE:
        _CACHE[key] = _build(n_core)
    return _CACHE[key]


def _host_weights(W1, b1, W2, b2):
    import ml_dtypes

    W1 = np.asarray(W1, np.float32)
    b1 = np.asarray(b1, np.float32)
    W2 = np.asarray(W2, np.float32)
    b2 = np.asarray(b2, np.float32)
    bf16 = ml_dtypes.bfloat16
    sign = np.where(np.arange(D) < SIGN, -1.0, 1.0).astype(np.float32)
    bcol = np.empty((128, 3), np.float32)
    bcol[:, 0:2] = b1.reshape(2, 128).T
    bcol[:, 2] = b2

    def as_words(a_bf16):
        return np.ascontiguousarray(a_bf16).reshape(128, 256).view(np.float32)

    wpack = np.concatenate([
        as_words(np.ascontiguousarray(W1.T).astype(bf16)),       # [0:128)
        as_words(np.ascontiguousarray(
            W2.T.reshape(2, 128, D).transpose(1, 0, 2)).astype(bf16)),
        as_words(np.ascontiguousarray(sign[:, None] * W2).astype(bf16)),
        as_words(np.ascontiguousarray(
            (W1 * sign[None, :]).reshape(2, 128, D).transpose(1, 0, 2)
        ).astype(bf16)),
        as_words(np.ascontiguousarray(
            (-2.0 * W2.T).reshape(2, 128, D).transpose(1, 0, 2)).astype(bf16)),
        bcol,
    ], axis=1)
    return {"wpack": np.ascontiguousarray(wpack)}


def _host_input(core_rows, n_core=N_CORE):
    """[n_core, 2D] f32 -> [nchunk, 128, 2, NF] bf16 feature-major."""
    import ml_dtypes

    nchunk = n_core // NF
    a = core_rows.reshape(nchunk, NF, 2, 128)     # [c, n, fb, j]
    a = a.transpose(0, 3, 2, 1)                   # [c, j, fb, n]
    return np.ascontiguousarray(a.astype(ml_dtypes.bfloat16))


def _run(inp_np, W1, b1, W2, b2, trace=False):
    import ml_dtypes
    from concourse.bass_utils import run_bass_kernel_spmd

    nc = _get_nc(N_CORE)
    wmap = _host_weights(W1, b1, W2, b2)
    in_maps = []
    for c in range(NCORES):
        m = dict(wmap)
        m["inp"] = _host_input(inp_np[c * N_CORE:(c + 1) * N_CORE])
        in_maps.append(m)
    res = run_bass_kernel_spmd(nc, in_maps, list(range(NCORES)), trace=trace)

    # outd[c, p, b, :] = dv row (c*NF + b*128 + p), bf16
    def widen(a):
        a = np.asarray(a)
        if a.dtype != ml_dtypes.bfloat16:
            a = a.view(ml_dtypes.bfloat16)
        return a.astype(np.float32)

    dv = np.concatenate(
        [widen(r["outd"]).transpose(0, 2, 1, 3).reshape(N_CORE, D)
         for r in res.results], axis=0)
    out = np.hstack([inp_np[:, D:TWO_D], dv])
    return np.ascontiguousarray(out), res


def kernel(t=None, input_=None, W1=None, b1=None, W2=None, b2=None, **kw):
    inp_np = np.ascontiguousarray(np.asarray(input_, np.float32))
    trace = bool(int(os.environ.get("KERNEL_TRACE", "0")))
    out, _ = _run(inp_np, W1, b1, W2, b2, trace=trace)
    return out


def run_traced(inputs):
    """Returns (out, exec_time_ns, trace_path). Used by test.py."""
    inp_np = np.ascontiguousarray(np.asarray(inputs["input_"], np.float32))
    out, res = _run(inp_np, inputs["W1"], inputs["b1"], inputs["W2"],
                    inputs["b2"], trace=True)
    trace_path = None
    if res.instructions_and_trace is not None:
        trace_path = res.instructions_and_trace[1]
    return out, res.exec_time_ns, trace_path


# revision 37
# speedup vs baseline: 1.0285x; 1.0285x over previous
"""Trainium2 raw-bass kernel for nn_Connection_v5 (geodesic-spray RHS).

Math (per sample n, D=128, 2D=256):
    x = input_[:, :D], v = input_[:, D:]
    z1 = x @ W1.T + b1 ; h = relu(z1) ; mask = z1 > 0
    s  = sigmoid(h @ W2.T + b2) ; sign_j = -1 if j < 4 else 1
    g  = (s + 0.618) * sign ; jac[i,j] = sign_i s_i(1-s_i) (W2 (mask*W1))[i,j]
    dv[j] = -1/g_j sum_i v_i^2 jac[i,j] + 2 v_j/g_j sum_i v_i jac[j,i]
    out = [v, dv]

Folded all-bf16 dataflow (host-validated ~7e-3 rel err vs the 2e-2 gate):
    [z1|u]^T = W1T @ [x|v]^T                    M1M3 (bf16, 512 moving)
    h = relu(z1+b1) ; s = sigmoid(z2+b2)        scalar ACT from PSUM
    nsps=(s-1)s ; wt=v^2*nsps ; gr=1/(s+.618) ; grw=gr*v*nsps
    at = (sign_i*W2)^T wt                       M4 (overwrites z1 in PSUM)
    mu = (h>0)*u ; am = (h>0)*at                two 3D DVE ops
    At'= (W1*sign_j) @ am ; Ct = (-2 W2^T) @ mu M5/M6 -> one PSUM tile
    dv = gr*At' + grw*Ct                        paired mult + add
    dv^T -> sample-major via PE transpose, stored bf16, widened on host

Implementation: raw bass (no TileContext).  Five hand-written in-order
engine programs in a 7-iteration software pipeline (4 chunks of 256
samples), manual semaphores, statically allocated per-chunk SBUF (no reuse
-> no WAR sems), hand-packed PSUM (8 banks exactly).  Inputs are
pre-transposed to feature-major bf16 on the host so each chunk's DMA lands
directly as a matmul moving operand.  The PE is warmed with junk matmuls
while the first input chunk + weights are in flight.  Ends with a single
block barrier + sem clear (vs Tile's ~10us drain butterfly).

Sharding: pure data-parallel over N=8192 across 8 cores; weights
replicated; the v half of the output never touches the device.
"""

import os
import numpy as np

D = 128
TWO_D = 256
N_TOTAL = 8192
NCORES = 8
N_CORE = N_TOTAL // NCORES  # 1024
NF = 256                    # samples per pipeline chunk
CONST = 0.618
SIGN = 4
N_WARM_MM = 6               # junk matmuls ramping PE during initial DMA wait

_CACHE = {}


def _build(n_core=N_CORE):
    from contextlib import ExitStack

    import concourse.bacc as bacc
    import concourse.mybir as mybir

    f32 = mybir.dt.float32
    bf16 = mybir.dt.bfloat16
    i32 = mybir.dt.int32
    Act = mybir.ActivationFunctionType
    Op = mybir.AluOpType

    nchunk = n_core // NF
    nb = NF // 128

    nc = bacc.Bacc("TRN2", target_bir_lowering=False, debug=False,
                   num_devices=NCORES)

    # inp[c, j, fb, n] = input row (c*NF + n), feature (fb*128 + j), bf16
    # feature-major (fb=0 -> x^T, fb=1 -> v^T), 1KB/partition/chunk.
    inp = nc.dram_tensor("inp", [nchunk, 128, 2, NF], bf16,
                         kind="ExternalInput").ap()
    wpack = nc.dram_tensor("wpack", [128, 643], f32,
                           kind="ExternalInput").ap()
    outd = nc.dram_tensor("outd", [nchunk, 128, nb, D], bf16,
                          kind="ExternalOutput").ap()

    NIT = nchunk + 3

    with ExitStack() as ctx:
        E = ctx.enter_context

        # --- SBUF (static; per-chunk copies -> no cross-chunk WAR) ---
        sb_pack = E(nc.sbuf_tensor("sb_pack", [128, 643], f32))
        warm = E(nc.sbuf_tensor("warm", [128, 1], f32))
        idn_i = E(nc.sbuf_tensor("idn_i", [128, 128], i32))
        idn_b = E(nc.sbuf_tensor("idn_b", [128, 128], bf16))
        junk = E(nc.sbuf_tensor("junk", [128, 512], bf16))
        xv = [E(nc.sbuf_tensor(f"xv{c}", [128, 2, NF], bf16))
              for c in range(nchunk)]
        h = [E(nc.sbuf_tensor(f"h{c}", [128, 2, NF], bf16))
             for c in range(nchunk)]
        s_ = [E(nc.sbuf_tensor(f"s{c}", [128, NF], bf16))
              for c in range(nchunk)]
        gs = [E(nc.sbuf_tensor(f"gs{c}", [128, NF], f32))
              for c in range(nchunk)]
        nsps = [E(nc.sbuf_tensor(f"nsps{c}", [128, NF], bf16))
                for c in range(nchunk)]
        v2 = [E(nc.sbuf_tensor(f"v2{c}", [128, NF], bf16))
              for c in range(nchunk)]
        wt = [E(nc.sbuf_tensor(f"wt{c}", [128, NF], bf16))
              for c in range(nchunk)]
        w = [E(nc.sbuf_tensor(f"w{c}", [128, NF], bf16))
             for c in range(nchunk)]
        grgw = [E(nc.sbuf_tensor(f"grgw{c}", [128, 2, NF], f32))
                for c in range(nchunk)]
        amu = [E(nc.sbuf_tensor(f"amu{c}", [128, 2, 2, NF], bf16))
               for c in range(nchunk)]
        prod = [E(nc.sbuf_tensor(f"prod{c}", [128, 2, NF], bf16))
                for c in range(nchunk)]
        dvT = [E(nc.sbuf_tensor(f"dvT{c}", [128, NF], bf16))
               for c in range(nchunk)]
        ob = [E(nc.sbuf_tensor(f"ob{c}", [128, nb, D], bf16))
              for c in range(nchunk)]

        # --- PSUM: 8 banks exactly ---
        zu = [E(nc.psum_tensor(f"zu{i}", [128, 2, 2, NF], f32))
              for i in range(2)]                            # 2 banks each
        z2 = E(nc.psum_tensor("z2", [128, NF], f32))        # 1 bank
        acp = E(nc.psum_tensor("ac", [128, 2, NF], f32))    # 1 bank
        dvp = E(nc.psum_tensor("dvp", [128, nb, D], bf16))  # 1 bank
        pat = E(nc.psum_tensor("pat", [128, 2, NF], f32))   # 1 bank (M4 out
        #     + PE warmup target; single-buffered, guarded by s_am)

        w1t = sb_pack[:, 0:128].bitcast(bf16)
        w2t = sb_pack[:, 128:256].bitcast(bf16).rearrange(
            "p (a b) -> p a b", a=2)
        w2sgn = sb_pack[:, 256:384].bitcast(bf16)
        w1p = sb_pack[:, 384:512].bitcast(bf16).rearrange(
            "p (a b) -> p a b", a=2)
        w2t2 = sb_pack[:, 512:640].bitcast(bf16).rearrange(
            "p (a b) -> p a b", a=2)
        bcol = sb_pack[:, 640:643]

        if True:
            S = {name: nc.alloc_semaphore(name) for name in
                 ["s_w1", "s_wr", "s_out",
                  "s_zu", "s_z2", "s_at", "s_ac", "s_dvp",
                  "s_h", "s_s", "s_gs", "s_ob",
                  "s_mu", "s_am", "s_nsps", "s_wt", "s_gr", "s_prod",
                  "s_v2", "s_grw", "s_dvT",
                  "s_iota", "s_jm", "s_idnb", "s_wm"]}
            s_in = [nc.alloc_semaphore(f"s_in{c}") for c in range(nchunk)]
            sem_nums = sorted(h.num for h in
                              list(S.values()) + list(s_in))
            sem_range = range(sem_nums[0], sem_nums[-1] + 1)

            NI = nchunk + 4

            with nc.Block(no_gpsimd_drain=True) as block:

                @block.sync
                def _(sp):
                    sp.dma_start(out=sb_pack[:, 0:128],
                                 in_=wpack[:, 0:128]).then_inc(S["s_w1"], 16)
                    sp.dma_start(out=xv[0][:], in_=inp[0]).then_inc(
                        s_in[0], 16)
                    sp.dma_start(out=sb_pack[:, 128:643],
                                 in_=wpack[:, 128:643]).then_inc(
                                     S["s_wr"], 16)
                    for c in range(1, nchunk):
                        sp.dma_start(out=xv[c][:], in_=inp[c]).then_inc(
                            s_in[c], 16)
                    for c in range(nchunk):
                        sp.wait_ge(S["s_ob"], c + 1)
                        sp.dma_start(out=outd[c], in_=ob[c][:]).then_inc(
                            S["s_out"], 16)
                    sp.wait_ge(S["s_out"], 16 * nchunk)

                @block.gpsimd
                def _(gp):
                    gp.iota(idn_i[:], pattern=[[1, 128]], base=0,
                            channel_multiplier=-1).then_inc(S["s_iota"])
                    gp.memset(junk[:], 0.0).then_inc(S["s_jm"])
                    for i in range(NI):
                        c = i - 1
                        if 0 <= c < nchunk:           # wt(c) = v2*nsps
                            gp.wait_ge(S["s_nsps"], c + 1)
                            gp.wait_ge(S["s_v2"], c + 1)
                            gp.tensor_tensor(wt[c][:], v2[c][:],
                                             nsps[c][:],
                                             Op.mult).then_inc(S["s_wt"])
                        if 0 <= c < nchunk:           # w(c) = v*nsps
                            gp.tensor_tensor(w[c][:], xv[c][:, 1, :],
                                             nsps[c][:], Op.mult)
                        c = i - 2
                        if 0 <= c < nchunk:           # dvT(c) = prod0+prod1
                            gp.wait_ge(S["s_prod"], c + 1)
                            gp.tensor_tensor(dvT[c][:], prod[c][:, 0, :],
                                             prod[c][:, 1, :],
                                             Op.add).then_inc(S["s_dvT"])
                        c = i - 1
                        if 0 <= c < nchunk:           # grw(c) = w*gr
                            gp.wait_ge(S["s_gr"], c + 1)
                            gp.drain()   # Pool pipe flush before reading w
                            gp.tensor_tensor(grgw[c][:, 1, :], w[c][:],
                                             grgw[c][:, 0, :],
                                             Op.mult).then_inc(S["s_grw"])

                @block.vector
                def _(ve):
                    ve.memset(warm[:], 0.0).then_inc(S["s_wm"])
                    ve.wait_ge(S["s_iota"], 1)
                    ve.tensor_scalar(out=idn_b[:], in0=idn_i[:], scalar1=0,
                                     scalar2=None,
                                     op0=Op.is_equal).then_inc(S["s_idnb"])
                    for i in range(NI):
                        c = i - 2
                        if 0 <= c < nchunk:           # am(c) = (h>0)*at
                            ve.wait_ge(S["s_at"], c + 1)
                            ve.scalar_tensor_tensor(
                                out=amu[c][:, :, 0, :], in0=h[c][:],
                                scalar=0.0, in1=pat[:],
                                op0=Op.is_gt,
                                op1=Op.mult).then_inc(S["s_am"])
                        c = i - 1
                        if 0 <= c < nchunk:
                            # nsps(c) = (s-1)*s
                            ve.wait_ge(S["s_s"], c + 1)
                            ve.scalar_tensor_tensor(
                                out=nsps[c][:], in0=s_[c][:], scalar=1.0,
                                in1=s_[c][:], op0=Op.subtract,
                                op1=Op.mult).then_inc(S["s_nsps"])
                            # gr(c) = 1/(s+0.618)
                            ve.wait_ge(S["s_gs"], c + 1)
                            ve.reciprocal_approx_fast(
                                out=grgw[c][:, 0, :],
                                in_=gs[c][:]).then_inc(S["s_gr"])
                        c = i - 2
                        if 0 <= c < nchunk:           # prod = grgw*[At|Ct]
                            ve.wait_ge(S["s_ac"], c + 1)
                            ve.wait_ge(S["s_grw"], c + 1)
                            ve.tensor_tensor(prod[c][:], grgw[c][:],
                                             acp[:],
                                             Op.mult).then_inc(S["s_prod"])
                        c = i
                        if 0 <= c < nchunk:           # mu(c) = (h>0)*u
                            ve.wait_ge(S["s_zu"], c + 1)
                            ve.wait_ge(S["s_h"], c + 1)
                            ve.scalar_tensor_tensor(
                                out=amu[c][:, :, 1, :], in0=h[c][:],
                                scalar=0.0, in1=zu[c % 2][:, :, 1, :],
                                op0=Op.is_gt,
                                op1=Op.mult).then_inc(S["s_mu"])

                @block.scalar
                def _(sc):
                    sc.wait_ge(S["s_wm"], 1)
                    sc.activation(out=warm[:], in_=warm[:], func=Act.Sigmoid,
                                  bias=0.0, scale=1.0)
                    sc.wait_ge(S["s_wr"], 16)   # bias columns loaded
                    for i in range(NI):
                        c = i - 4
                        if 0 <= c < nchunk:           # ob(c)
                            sc.wait_ge(S["s_dvp"], c + 1)
                            sc.copy(out=ob[c][:],
                                    in_=dvp[:]).then_inc(S["s_ob"])
                        c = i
                        if 0 <= c < nchunk:           # v2(c) = v^2 (Square)
                            sc.wait_ge(s_in[c], 16)
                            sc.activation(out=v2[c][:], in_=xv[c][:, 1, :],
                                          func=Act.Square, bias=0.0,
                                          scale=1.0).then_inc(S["s_v2"])
                        c = i - 1
                        if 0 <= c < nchunk:           # sigmoid(c), gs(c)
                            sc.wait_ge(S["s_z2"], c + 1)
                            sc.activation(out=s_[c][:], in_=z2[:],
                                          func=Act.Sigmoid,
                                          bias=bcol[:, 2:3],
                                          scale=1.0).then_inc(S["s_s"])
                            sc.drain()   # ACT pipe flush before reading s
                            sc.activation(out=gs[c][:], in_=s_[c][:],
                                          func=Act.Copy, bias=CONST,
                                          scale=1.0).then_inc(S["s_gs"])
                        c = i
                        if 0 <= c < nchunk:           # relu(c)
                            sc.wait_ge(S["s_zu"], c + 1)
                            for k in range(2):
                                ins = sc.activation(
                                    out=h[c][:, k, :],
                                    in_=zu[c % 2][:, k, 0, :],
                                    func=Act.Relu,
                                    bias=bcol[:, k:k + 1], scale=1.0)
                            ins.then_inc(S["s_h"])

                @block.tensor
                def _(pe):
                    pe.wait_ge(S["s_idnb"], 1)
                    pe.wait_ge(S["s_jm"], 1)
                    for wi in range(N_WARM_MM):
                        nc.tensor.matmul(
                            pat[:].rearrange("p a b -> p (a b)"),
                            idn_b[:], junk[:], start=True, stop=True)
                    pe.wait_ge(S["s_wr"], 16)
                    for i in range(NI):
                        c = i - 2
                        if 0 <= c < nchunk:           # M4(c): at = W2sgn @ wt
                            pe.wait_ge(S["s_wt"], c + 1)
                            if c >= 1:                # pat bank reuse
                                pe.wait_ge(S["s_am"], c)
                            for k in range(2):
                                ins = nc.tensor.matmul(
                                    pat[:, k, :],
                                    w2sgn[:, 128 * k:128 * (k + 1)],
                                    wt[c][:], start=True, stop=True)
                            ins.then_inc(S["s_at"])
                        c = i - 1
                        if 0 <= c < nchunk:           # M2(c)
                            pe.wait_ge(S["s_h"], c + 1)
                            if c >= 1:                # z2 bank reuse
                                pe.wait_ge(S["s_s"], c)
                            for k in range(2):
                                ins = nc.tensor.matmul(
                                    z2[:], w2t[:, k, :], h[c][:, k, :],
                                    start=(k == 0), stop=(k == 1))
                            ins.then_inc(S["s_z2"])
                        c = i
                        if 0 <= c < nchunk:           # M1M3(c)
                            pe.wait_ge(s_in[c], 16)
                            if c == 0:
                                pe.wait_ge(S["s_w1"], 16)
                            if c >= 2:                # zu slot reuse
                                pe.wait_ge(S["s_mu"], c - 1)
                                pe.wait_ge(S["s_h"], c - 1)
                            mov = xv[c][:].rearrange("p a b -> p (a b)")
                            for k in range(2):
                                ins = nc.tensor.matmul(
                                    zu[c % 2][:, k, :, :].rearrange(
                                        "p a b -> p (a b)"),
                                    w1t[:, 128 * k:128 * (k + 1)],
                                    mov, start=True, stop=True)
                            ins.then_inc(S["s_zu"])
                        c = i - 2
                        if 0 <= c < nchunk:           # M5(c), M6(c)
                            pe.wait_ge(S["s_am"], c + 1)
                            pe.wait_ge(S["s_mu"], c + 1)
                            if c >= 1:                # ac bank reuse
                                pe.wait_ge(S["s_prod"], c)
                            for k in range(2):
                                nc.tensor.matmul(
                                    acp[:, 0, :], w1p[:, k, :],
                                    amu[c][:, k, 0, :],
                                    start=(k == 0), stop=(k == 1))
                            for k in range(2):
                                ins = nc.tensor.matmul(
                                    acp[:, 1, :], w2t2[:, k, :],
                                    amu[c][:, k, 1, :],
                                    start=(k == 0), stop=(k == 1))
                            ins.then_inc(S["s_ac"])
                        c = i - 3
                        if 0 <= c < nchunk:           # trout(c)
                            pe.wait_ge(S["s_dvT"], c + 1)
                            if c >= 1:                # dv bank reuse
                                pe.wait_ge(S["s_ob"], c)
                            for b in range(nb):
                                ins = nc.tensor.transpose(
                                    dvp[:, b, :],
                                    dvT[c][:, 128 * b:128 * (b + 1)],
                                    idn_b[:])
                            ins.then_inc(S["s_dvp"])

            # after the block-exit barrier: zero our sems so a re-execution
            # of this NEFF (or a later kernel) starts clean.  Cheap
            # (~0.2us) vs Tile's dma_reset + double-barrier butterfly.
            nc.gpsimd.sem_clear(sem_range)

    nc.compile()
    return nc


def _get_nc(n_core=N_CORE):
    key = ("nc", n_core)
    if key not in _CACHE:
        _CACHE[key] = _build(n_core)
    return _CACHE[key]


def _host_weights(W1, b1, W2, b2):
    import ml_dtypes

    W1 = np.asarray(W1, np.float32)
    b1 = np.asarray(b1, np.float32)
    W2 = np.asarray(W2, np.float32)
    b2 = np.asarray(b2, np.float32)
    bf16 = ml_dtypes.bfloat16
    sign = np.where(np.arange(D) < SIGN, -1.0, 1.0).astype(np.float32)
    bcol = np.empty((128, 3), np.float32)
    bcol[:, 0:2] = b1.reshape(2, 128).T
    bcol[:, 2] = b2

    def as_words(a_bf16):
        return np.ascontiguousarray(a_bf16).reshape(128, 256).view(np.float32)

    wpack = np.concatenate([
        as_words(np.ascontiguousarray(W1.T).astype(bf16)),       # [0:128)
        as_words(np.ascontiguousarray(
            W2.T.reshape(2, 128, D).transpose(1, 0, 2)).astype(bf16)),
        as_words(np.ascontiguousarray(sign[:, None] * W2).astype(bf16)),
        as_words(np.ascontiguousarray(
            (W1 * sign[None, :]).reshape(2, 128, D).transpose(1, 0, 2)
        ).astype(bf16)),
        as_words(np.ascontiguousarray(
            (-2.0 * W2.T).reshape(2, 128, D).transpose(1, 0, 2)).astype(bf16)),
        bcol,
    ], axis=1)
    return {"wpack": np.ascontiguousarray(wpack)}


def _host_input(core_rows, n_core=N_CORE):
    """[n_core, 2D] f32 -> [nchunk, 128, 2, NF] bf16 feature-major."""
    import ml_dtypes

    nchunk = n_core // NF
    a = core_rows.reshape(nchunk, NF, 2, 128)     # [c, n, fb, j]
    a = a.transpose(0, 3, 2, 1)                   # [c, j, fb, n]
    return np.ascontiguousarray(a.astype(ml_dtypes.bfloat16))


def _run(inp_np, W1, b1, W2, b2, trace=False):
    import ml_dtypes
    from concourse.bass_utils import run_bass_kernel_spmd

    nc = _get_nc(N_CORE)
    wmap = _host_weights(W1, b1, W2, b2)
    in_maps = []
    for c in range(NCORES):
        m = dict(wmap)
        m["inp"] = _host_input(inp_np[c * N_CORE:(c + 1) * N_CORE])
        in_maps.append(m)
    res = run_bass_kernel_spmd(nc, in_maps, list(range(NCORES)), trace=trace)

    # outd[c, p, b, :] = dv row (c*NF + b*128 + p), bf16
    def widen(a):
        a = np.asarray(a)
        if a.dtype != ml_dtypes.bfloat16:
            a = a.view(ml_dtypes.bfloat16)
        return a.astype(np.float32)

    dv = np.concatenate(
        [widen(r["outd"]).transpose(0, 2, 1, 3).reshape(N_CORE, D)
         for r in res.results], axis=0)
    out = np.hstack([inp_np[:, D:TWO_D], dv])
    return np.ascontiguousarray(out), res


def kernel(t=None, input_=None, W1=None, b1=None, W2=None, b2=None, **kw):
    inp_np = np.ascontiguousarray(np.asarray(input_, np.float32))
    trace = bool(int(os.environ.get("KERNEL_TRACE", "0")))
    out, _ = _run(inp_np, W1, b1, W2, b2, trace=trace)
    return out


def run_traced(inputs):
    """Returns (out, exec_time_ns, trace_path). Used by test.py."""
    inp_np = np.ascontiguousarray(np.asarray(inputs["input_"], np.float32))
    out, res = _run(inp_np, inputs["W1"], inputs["b1"], inputs["W2"],
                    inputs["b2"], trace=True)
    trace_path = None
    if res.instructions_and_trace is not None:
        trace_path = res.instructions_and_trace[1]
    return out, res.exec_time_ns, trace_path
